# revision 12
# baseline (speedup 1.0000x reference)
"""Batched Bjorck orthogonalization on 8 TRN2 NeuronCores.

w: [64, 1024, 1024] f32. 13 iterations of W <- 1.5 W - 0.5 W (W^T W).
Sharding: batch dim across 8 cores (8 matrices per core), fully independent.

Math: tf32x3 (fp32r hi/lo Dekker split) matmuls on the PE array:
  X*Y ~= Xhi*Yhi + Xhi*Ylo + Xlo*Yhi   (fp32r = e8m11, PSUM accumulates fp32)
which reproduces fp32-class precision (~2e-7 vs the fp32 reference) at
3 cycles/row vs 4 for native fp32 matmul.

Per-core per-matrix iteration (all on-chip, state in SBUF):
  G: A' = 1.5 I - 0.5 (W^T W)   [384 matmuls -> PSUM; scale folded into drain]
  U: W  = W A'                  [384 matmuls; hi/lo split overwrites state]
  T: refresh W^T hi/lo          [128 PE transposes]
"""

import numpy as np

_NC_CACHE = {}

P = 128  # partitions
NMAT = 1024  # matrix dim
C = 8  # row chunks (NMAT / P)
FB = 512  # psum free-block width
NB = 2  # free blocks per 1024 (NMAT / FB)
ITERS = 13


def _build(B, iters=ITERS, static=False):
    import concourse.bacc as bacc
    import concourse.bass as bass
    import concourse.mybir as mybir
    from concourse.tile import TileContext

    F32 = mybir.dt.float32
    F32R = mybir.dt.float32r
    COPY = mybir.ActivationFunctionType.Copy
    SUB = mybir.AluOpType.subtract
    ADD = mybir.AluOpType.add

    nc = bacc.Bacc("TRN2", target_bir_lowering=False, debug=False)
    w = nc.dram_tensor("w", [B, NMAT, NMAT], F32, kind="ExternalInput")
    o = nc.dram_tensor("o", [B, NMAT, NMAT], F32, kind="ExternalOutput")
    eye = nc.dram_tensor("eye", [P, P], F32, kind="ExternalInput")  # 1.5*I
    ide = nc.dram_tensor("ide", [P, P], F32, kind="ExternalInput")  # I

    with TileContext(nc) as tc:
        with (
            tc.tile_pool(name="state", bufs=1) as st,
            tc.tile_pool(name="const", bufs=1) as cn,
            tc.tile_pool(name="tmp", bufs=2) as tp,
            tc.tile_pool(name="pg", bufs=3, space="PSUM") as pg,
            tc.tile_pool(name="pu", bufs=3, space="PSUM") as pu,
            tc.tile_pool(name="pt", bufs=2, space="PSUM") as pt,
        ):
            eye_s = cn.tile([P, P], F32, tag="eye")
            ide32 = cn.tile([P, P], F32, tag="ide32")
            ide_r = cn.tile([P, P], F32R, tag="ide_r")
            nc.sync.dma_start(eye_s[:], eye.ap())
            nc.sync.dma_start(ide32[:], ide.ap())
            nc.scalar.activation(ide_r[:], ide32[:], COPY)

            from contextlib import nullcontext

            loop_cm = nullcontext(0) if static else tc.For_i(0, B)
            with loop_cm as ib:
              for _sib in range(B if static else 1):
                if static:
                    ib = _sib
                WH = st.tile([P, C * NMAT], F32R, tag="WH")
                WL = st.tile([P, C * NMAT], F32R, tag="WL")
                WHT = st.tile([P, C * NMAT], F32R, tag="WHT")
                WLT = st.tile([P, C * NMAT], F32R, tag="WLT")
                AH = st.tile([P, C * NMAT], F32R, tag="AH")
                AL = st.tile([P, C * NMAT], F32R, tag="AL")
                AH32 = AH[:].bitcast(F32)
                AL32 = AL[:].bitcast(F32)
                WH32 = WH[:].bitcast(F32)
                WL32 = WL[:].bitcast(F32)
                WHT32 = WHT[:].bitcast(F32)

                # ---- load W (fp32) block-wise via tmp tiles, split -> WH + WL
                for c in range(C):
                    for nb2 in range(NB):
                        s = slice(c * NMAT + nb2 * FB, c * NMAT + (nb2 + 1) * FB)
                        t32 = tp.tile([P, FB], F32, tag="t32")
                        nc.sync.dma_start(
                            t32[:],
                            w.ap()[
                                bass.ds(ib, 1),
                                c * P : (c + 1) * P,
                                nb2 * FB : (nb2 + 1) * FB,
                            ],
                        )
                        t32b = tp.tile([P, FB], F32, tag="t32b")
                        nc.scalar.activation(WH[:, s], t32[:], COPY)
                        nc.vector.tensor_tensor(t32b[:], t32[:], WH32[:, s], SUB)
                        nc.scalar.activation(WL[:, s], t32b[:], COPY)

                def phase_T():
                    # WHT/WLT[dst_c] = transpose of column-block dst_c of WH/WL
                    for src, dst in ((WH, WHT), (WL, WLT)):
                        for dc in range(C):
                            for half in range(NB):
                                ptile = pt.tile([P, FB], F32R, tag="pt")
                                for q in range(4):
                                    i = half * 4 + q  # source row-chunk
                                    nc.tensor.transpose(
                                        ptile[:, q * P : (q + 1) * P],
                                        src[:, i * NMAT + dc * P : i * NMAT + (dc + 1) * P],
                                        ide_r[:],
                                    )
                                nc.scalar.activation(
                                    dst[:, dc * NMAT + half * FB : dc * NMAT + (half + 1) * FB],
                                    ptile[:],
                                    COPY,
                                )

                phase_T()

                # Gram triangle plan: tile (m, nb2) covers A' block-cols
                # [4*nb2, 4*nb2+4). nlo = # leading blocks strictly below the
                # diagonal; skip tile if all 4 are, trim N when 1-2 are (N>=256
                # keeps fp32r at 1 cyc/row), keep full when 3 (N=128 is slow).
                def g_plan():
                    plan = []  # (m, nb2, nlo, n_eff)
                    computed = set()  # (mb, nb) 128-blocks present
                    for m in range(C):
                        for nb2 in range(NB):
                            nlo = m - 4 * nb2
                            if nlo >= 4:
                                continue
                            if nlo not in (1, 2):
                                nlo = 0
                            plan.append((m, nb2, nlo, FB - nlo * P))
                            for n in range(4 * nb2 + nlo, 4 * nb2 + 4):
                                computed.add((m, n))
                    recon = []  # (mb, [contiguous nb list])
                    for mb in range(C):
                        run = []
                        for nb in range(C):
                            if nb < mb and (mb, nb) not in computed:
                                run.append(nb)
                            else:
                                if run:
                                    recon.append((mb, run))
                                run = []
                        if run:
                            recon.append((mb, run))
                    recon = [
                        (mb, run[i : i + 4])
                        for mb, run in recon
                        for i in range(0, len(run), 4)
                    ]
                    return plan, recon

                G_PLAN, G_RECON = g_plan()

                for it in range(iters):
                    last = it == iters - 1
                    # ---- G: A' = 1.5 I - 0.5 W^T W (upper triangle + diag)
                    for m, nb2, nlo, n_eff in G_PLAN:
                        g = pg.tile([P, FB], F32, tag="pg")
                        moff = nb2 * FB + nlo * P
                        for k in range(C):
                            sh = WH[:, k * NMAT + m * P : k * NMAT + (m + 1) * P]
                            sl = WL[:, k * NMAT + m * P : k * NMAT + (m + 1) * P]
                            mh = WH[:, k * NMAT + moff : k * NMAT + moff + n_eff]
                            ml = WL[:, k * NMAT + moff : k * NMAT + moff + n_eff]
                            nc.tensor.matmul(g[:, :n_eff], sh, mh, start=(k == 0), stop=False)
                            nc.tensor.matmul(g[:, :n_eff], sh, ml, start=False, stop=False)
                            nc.tensor.matmul(g[:, :n_eff], sl, mh, start=False, stop=(k == C - 1))
                        t32 = tp.tile([P, FB], F32, tag="t32")
                        nc.scalar.activation(t32[:, :n_eff], g[:, :n_eff], COPY, scale=-0.5)
                        if m // 4 == nb2:  # diag block at kept-offset (m%4 - nlo)
                            doff = (m % 4 - nlo) * P
                            nc.vector.tensor_tensor(
                                t32[:, doff : doff + P], t32[:, doff : doff + P], eye_s[:], ADD
                            )
                        d = slice(m * NMAT + moff, m * NMAT + moff + n_eff)
                        t32b = tp.tile([P, FB], F32, tag="t32b")
                        nc.scalar.activation(AH[:, d], t32[:, :n_eff], COPY)
                        nc.vector.tensor_tensor(t32b[:, :n_eff], t32[:, :n_eff], AH32[:, d], SUB)
                        nc.scalar.activation(AL[:, d], t32b[:, :n_eff], COPY)
                    # ---- G recon: lower blocks = transpose of upper blocks
                    for src_t, dst_t in ((AH, AH), (AL, AL)):
                        for mb, nbs in G_RECON:
                            n_r = len(nbs) * P
                            pr = pt.tile([P, FB], F32R, tag="pt")
                            for qi, nb in enumerate(nbs):
                                nc.tensor.transpose(
                                    pr[:, qi * P : (qi + 1) * P],
                                    src_t[:, nb * NMAT + mb * P : nb * NMAT + (mb + 1) * P],
                                    ide_r[:],
                                )
                            nc.scalar.activation(
                                dst_t[:, mb * NMAT + nbs[0] * P : mb * NMAT + nbs[0] * P + n_r],
                                pr[:, :n_r],
                                COPY,
                            )
                    # ---- U: W = W A'
                    for i in range(C):
                        for nb2 in range(NB):
                            u = pu.tile([P, FB], F32, tag="pu")
                            for j in range(C):
                                sh = WHT[:, j * NMAT + i * P : j * NMAT + (i + 1) * P]
                                sl = WLT[:, j * NMAT + i * P : j * NMAT + (i + 1) * P]
                                mh = AH[:, j * NMAT + nb2 * FB : j * NMAT + (nb2 + 1) * FB]
                                ml = AL[:, j * NMAT + nb2 * FB : j * NMAT + (nb2 + 1) * FB]
                                nc.tensor.matmul(u[:], sh, mh, start=(j == 0), stop=False)
                                nc.tensor.matmul(u[:], sh, ml, start=False, stop=False)
                                nc.tensor.matmul(u[:], sl, mh, start=False, stop=(j == C - 1))
                            if last:
                                t32o = tp.tile([P, FB], F32, tag="t32o")
                                nc.scalar.activation(t32o[:], u[:], COPY)
                                nc.sync.dma_start(
                                    o.ap()[
                                        bass.ds(ib, 1),
                                        i * P : (i + 1) * P,
                                        nb2 * FB : (nb2 + 1) * FB,
                                    ],
                                    t32o[:],
                                )
                            else:
                                d = slice(i * NMAT + nb2 * FB, i * NMAT + (nb2 + 1) * FB)
                                t32b = tp.tile([P, FB], F32, tag="t32b")
                                nc.scalar.activation(WH[:, d], u[:], COPY)
                                nc.vector.tensor_tensor(t32b[:], u[:], WH32[:, d], SUB)
                                nc.scalar.activation(WL[:, d], t32b[:], COPY)
                    if not last:
                        phase_T()
    nc.compile()
    return nc


def _get_nc(B, iters=ITERS):
    key = (B, iters)
    if key not in _NC_CACHE:
        _NC_CACHE[key] = _build(B, iters)
    return _NC_CACHE[key]


def kernel(w) -> np.ndarray:
    from concourse.bass_utils import run_bass_kernel_spmd

    w = np.ascontiguousarray(np.asarray(w, dtype=np.float32))
    assert w.shape == (64, NMAT, NMAT), w.shape
    B = 8  # matrices per core
    nc = _get_nc(B)
    eye15 = (1.5 * np.eye(P)).astype(np.float32)
    ide = np.eye(P, dtype=np.float32)
    in_maps = [
        {"w": np.ascontiguousarray(w[c * B : (c + 1) * B]), "eye": eye15, "ide": ide}
        for c in range(8)
    ]
    res = run_bass_kernel_spmd(nc, in_maps, core_ids=list(range(8)))
    return np.concatenate([res.results[c]["o"] for c in range(8)], axis=0)


# revision 13
# speedup vs baseline: 1307.3304x; 1307.3304x over previous
"""Batched Bjorck orthogonalization on 8 TRN2 NeuronCores.

w: [64, 1024, 1024] f32. 13 iterations of W <- 1.5 W - 0.5 W (W^T W).
Sharding: batch dim across 8 cores (8 matrices per core), fully independent.

Math: tf32x3 (fp32r hi/lo Dekker split) matmuls on the PE array:
  X*Y ~= Xhi*Yhi + Xhi*Ylo + Xlo*Yhi   (fp32r = e8m11, PSUM accumulates fp32)
which reproduces fp32-class precision (~2e-7 vs the fp32 reference) at
3 cycles/row vs 4 for native fp32 matmul.

Per-core per-matrix iteration (all on-chip, state in SBUF):
  G: A' = 1.5 I - 0.5 (W^T W)   [384 matmuls -> PSUM; scale folded into drain]
  U: W  = W A'                  [384 matmuls; hi/lo split overwrites state]
  T: refresh W^T hi/lo          [128 PE transposes]
"""

import numpy as np

_NC_CACHE = {}

P = 128  # partitions
NMAT = 1024  # matrix dim
C = 8  # row chunks (NMAT / P)
FB = 512  # psum free-block width
NB = 2  # free blocks per 1024 (NMAT / FB)
ITERS = 13
PG_BUFS = 3
PU_BUFS = 3
PT_BUFS = 2


def _build(B, iters=ITERS, static=False):
    import concourse.bacc as bacc
    import concourse.bass as bass
    import concourse.mybir as mybir
    from concourse.tile import TileContext

    F32 = mybir.dt.float32
    F32R = mybir.dt.float32r
    COPY = mybir.ActivationFunctionType.Copy
    SUB = mybir.AluOpType.subtract
    ADD = mybir.AluOpType.add

    nc = bacc.Bacc("TRN2", target_bir_lowering=False, debug=False)
    w = nc.dram_tensor("w", [B, NMAT, NMAT], F32, kind="ExternalInput")
    o = nc.dram_tensor("o", [B, NMAT, NMAT], F32, kind="ExternalOutput")
    eye = nc.dram_tensor("eye", [P, P], F32, kind="ExternalInput")  # 1.5*I
    ide = nc.dram_tensor("ide", [P, P], F32, kind="ExternalInput")  # I

    with TileContext(nc) as tc:
        with (
            tc.tile_pool(name="state", bufs=1) as st,
            tc.tile_pool(name="const", bufs=1) as cn,
            tc.tile_pool(name="tmp", bufs=2) as tp,
            tc.tile_pool(name="pg", bufs=PG_BUFS, space="PSUM") as pg,
            tc.tile_pool(name="pu", bufs=PU_BUFS, space="PSUM") as pu,
            tc.tile_pool(name="pt", bufs=PT_BUFS, space="PSUM") as pt,
        ):
            eye_s = cn.tile([P, P], F32, tag="eye")
            ide32 = cn.tile([P, P], F32, tag="ide32")
            ide_r = cn.tile([P, P], F32R, tag="ide_r")
            nc.sync.dma_start(eye_s[:], eye.ap())
            nc.sync.dma_start(ide32[:], ide.ap())
            nc.scalar.activation(ide_r[:], ide32[:], COPY)

            from contextlib import nullcontext

            loop_cm = nullcontext(0) if static else tc.For_i(0, B)
            with loop_cm as ib:
              for _sib in range(B if static else 1):
                if static:
                    ib = _sib
                WH = st.tile([P, C * NMAT], F32R, tag="WH")
                WL = st.tile([P, C * NMAT], F32R, tag="WL")
                WHT = st.tile([P, C * NMAT], F32R, tag="WHT")
                WLT = st.tile([P, C * NMAT], F32R, tag="WLT")
                AH = st.tile([P, C * NMAT], F32R, tag="AH")
                AL = st.tile([P, C * NMAT], F32R, tag="AL")
                AH32 = AH[:].bitcast(F32)
                AL32 = AL[:].bitcast(F32)
                WH32 = WH[:].bitcast(F32)
                WL32 = WL[:].bitcast(F32)
                WHT32 = WHT[:].bitcast(F32)

                # ---- load W (fp32) block-wise via tmp tiles, split -> WH + WL
                for c in range(C):
                    for nb2 in range(NB):
                        s = slice(c * NMAT + nb2 * FB, c * NMAT + (nb2 + 1) * FB)
                        t32 = tp.tile([P, FB], F32, tag="t32")
                        nc.sync.dma_start(
                            t32[:],
                            w.ap()[
                                bass.ds(ib, 1),
                                c * P : (c + 1) * P,
                                nb2 * FB : (nb2 + 1) * FB,
                            ],
                        )
                        t32b = tp.tile([P, FB], F32, tag="t32b")
                        nc.scalar.activation(WH[:, s], t32[:], COPY)
                        nc.vector.tensor_tensor(t32b[:], t32[:], WH32[:, s], SUB)
                        nc.scalar.activation(WL[:, s], t32b[:], COPY)

                def phase_T():
                    # WHT/WLT[dst_c] = transpose of column-block dst_c of WH/WL
                    for src, dst in ((WH, WHT), (WL, WLT)):
                        for dc in range(C):
                            for half in range(NB):
                                ptile = pt.tile([P, FB], F32R, tag="pt")
                                for q in range(4):
                                    i = half * 4 + q  # source row-chunk
                                    nc.tensor.transpose(
                                        ptile[:, q * P : (q + 1) * P],
                                        src[:, i * NMAT + dc * P : i * NMAT + (dc + 1) * P],
                                        ide_r[:],
                                    )
                                nc.scalar.activation(
                                    dst[:, dc * NMAT + half * FB : dc * NMAT + (half + 1) * FB],
                                    ptile[:],
                                    COPY,
                                )

                phase_T()

                # Gram triangle plan: tile (m, nb2) covers A' block-cols
                # [4*nb2, 4*nb2+4). nlo = # leading blocks strictly below the
                # diagonal; skip tile if all 4 are, trim N when 1-2 are (N>=256
                # keeps fp32r at 1 cyc/row), keep full when 3 (N=128 is slow).
                def g_plan():
                    plan = []  # (m, nb2, nlo, n_eff)
                    computed = set()  # (mb, nb) 128-blocks present
                    for m in range(C):
                        for nb2 in range(NB):
                            nlo = m - 4 * nb2
                            if nlo >= 4:
                                continue
                            if nlo not in (1, 2):
                                nlo = 0
                            plan.append((m, nb2, nlo, FB - nlo * P))
                            for n in range(4 * nb2 + nlo, 4 * nb2 + 4):
                                computed.add((m, n))
                    recon = []  # (mb, [contiguous nb list])
                    for mb in range(C):
                        run = []
                        for nb in range(C):
                            if nb < mb and (mb, nb) not in computed:
                                run.append(nb)
                            else:
                                if run:
                                    recon.append((mb, run))
                                run = []
                        if run:
                            recon.append((mb, run))
                    recon = [
                        (mb, run[i : i + 4])
                        for mb, run in recon
                        for i in range(0, len(run), 4)
                    ]
                    return plan, recon

                G_PLAN, G_RECON = g_plan()

                for it in range(iters):
                    last = it == iters - 1
                    # ---- G: A' = 1.5 I - 0.5 W^T W (upper triangle + diag)
                    for m, nb2, nlo, n_eff in G_PLAN:
                        g = pg.tile([P, FB], F32, tag="pg")
                        moff = nb2 * FB + nlo * P
                        for k in range(C):
                            sh = WH[:, k * NMAT + m * P : k * NMAT + (m + 1) * P]
                            sl = WL[:, k * NMAT + m * P : k * NMAT + (m + 1) * P]
                            mh = WH[:, k * NMAT + moff : k * NMAT + moff + n_eff]
                            ml = WL[:, k * NMAT + moff : k * NMAT + moff + n_eff]
                            nc.tensor.matmul(g[:, :n_eff], sh, mh, start=(k == 0), stop=False)
                            nc.tensor.matmul(g[:, :n_eff], sh, ml, start=False, stop=False)
                            nc.tensor.matmul(g[:, :n_eff], sl, mh, start=False, stop=(k == C - 1))
                        t32 = tp.tile([P, FB], F32, tag="t32")
                        nc.scalar.activation(t32[:, :n_eff], g[:, :n_eff], COPY, scale=-0.5)
                        if m // 4 == nb2:  # diag block at kept-offset (m%4 - nlo)
                            doff = (m % 4 - nlo) * P
                            nc.vector.tensor_tensor(
                                t32[:, doff : doff + P], t32[:, doff : doff + P], eye_s[:], ADD
                            )
                        d = slice(m * NMAT + moff, m * NMAT + moff + n_eff)
                        t32b = tp.tile([P, FB], F32, tag="t32b")
                        nc.scalar.activation(AH[:, d], t32[:, :n_eff], COPY)
                        nc.vector.tensor_tensor(t32b[:, :n_eff], t32[:, :n_eff], AH32[:, d], SUB)
                        nc.scalar.activation(AL[:, d], t32b[:, :n_eff], COPY)
                    # ---- G recon: lower blocks = transpose of upper blocks
                    for src_t, dst_t in ((AH, AH), (AL, AL)):
                        for mb, nbs in G_RECON:
                            n_r = len(nbs) * P
                            pr = pt.tile([P, FB], F32R, tag="pt")
                            for qi, nb in enumerate(nbs):
                                nc.tensor.transpose(
                                    pr[:, qi * P : (qi + 1) * P],
                                    src_t[:, nb * NMAT + mb * P : nb * NMAT + (mb + 1) * P],
                                    ide_r[:],
                                )
                            nc.scalar.activation(
                                dst_t[:, mb * NMAT + nbs[0] * P : mb * NMAT + nbs[0] * P + n_r],
                                pr[:, :n_r],
                                COPY,
                            )
                    # ---- U: W = W A'
                    for i in range(C):
                        for nb2 in range(NB):
                            u = pu.tile([P, FB], F32, tag="pu")
                            for j in range(C):
                                sh = WHT[:, j * NMAT + i * P : j * NMAT + (i + 1) * P]
                                sl = WLT[:, j * NMAT + i * P : j * NMAT + (i + 1) * P]
                                mh = AH[:, j * NMAT + nb2 * FB : j * NMAT + (nb2 + 1) * FB]
                                ml = AL[:, j * NMAT + nb2 * FB : j * NMAT + (nb2 + 1) * FB]
                                nc.tensor.matmul(u[:], sh, mh, start=(j == 0), stop=False)
                                nc.tensor.matmul(u[:], sh, ml, start=False, stop=False)
                                nc.tensor.matmul(u[:], sl, mh, start=False, stop=(j == C - 1))
                            if last:
                                t32o = tp.tile([P, FB], F32, tag="t32o")
                                nc.scalar.activation(t32o[:], u[:], COPY)
                                nc.sync.dma_start(
                                    o.ap()[
                                        bass.ds(ib, 1),
                                        i * P : (i + 1) * P,
                                        nb2 * FB : (nb2 + 1) * FB,
                                    ],
                                    t32o[:],
                                )
                            else:
                                d = slice(i * NMAT + nb2 * FB, i * NMAT + (nb2 + 1) * FB)
                                t32b = tp.tile([P, FB], F32, tag="t32b")
                                nc.scalar.activation(WH[:, d], u[:], COPY)
                                nc.vector.tensor_tensor(t32b[:], u[:], WH32[:, d], SUB)
                                nc.scalar.activation(WL[:, d], t32b[:], COPY)
                    if not last:
                        phase_T()
    nc.compile()
    return nc


def _get_nc(B, iters=ITERS):
    key = (B, iters)
    if key not in _NC_CACHE:
        _NC_CACHE[key] = _build(B, iters)
    return _NC_CACHE[key]


def kernel(w) -> np.ndarray:
    from concourse.bass_utils import run_bass_kernel_spmd

    w = np.ascontiguousarray(np.asarray(w, dtype=np.float32))
    assert w.shape == (64, NMAT, NMAT), w.shape
    B = 8  # matrices per core
    nc = _get_nc(B)
    eye15 = (1.5 * np.eye(P)).astype(np.float32)
    ide = np.eye(P, dtype=np.float32)
    in_maps = [
        {"w": np.ascontiguousarray(w[c * B : (c + 1) * B]), "eye": eye15, "ide": ide}
        for c in range(8)
    ]
    res = run_bass_kernel_spmd(nc, in_maps, core_ids=list(range(8)))
    return np.concatenate([res.results[c]["o"] for c in range(8)], axis=0)
